# revision 2
# baseline (speedup 1.0000x reference)
"""Trainium2 Bass kernel: dynamic deformable propagation (6 iterations).

v3: truncated 3x3 merged stencil (~9.6e-3 rel err vs the 2e-2 gate),
C fields SBUF-resident, three row-shifted F variants kept by small
SBUF DMAs (compute engines require partition start 0), conv as
single-shot [121->81] matmuls (dx-shift and bias folded into the
contraction -> no PSUM accumulation, 1 LDW+MM pair per triple),
host-side dtype staging (bf16 guidance / f16 fields -> no casting
DMAs on the Pool engine), flat per-iteration gate-field staging (124
fat descriptors per DMA), engine-partitioned iteration subtrees.

Sharding: one core per (image, x-half): 480 rows, 320 own cols +
redundant stale halo. Layout: rows on partitions (partition p of
block b = image row 124b+p), 4 row-blocks folded along the free dim.
"""
import sys, types

sys.path.insert(0, '/opt/trn_rl_repo')
import numpy as np


def _install_hook():
    try:
        import antenv
        if not hasattr(antenv, 'axon_hooks'):
            mod = types.ModuleType("antenv.axon_hooks")
            _h = [None]
            mod.set_axon_ntff_profile_hook = lambda h: _h.__setitem__(0, h)
            mod.get_axon_ntff_profile_hook = lambda: _h[0]
            sys.modules["antenv.axon_hooks"] = mod
            antenv.axon_hooks = mod
            from trn_agent_boot.trn_boot import _ntff_profile_via_ctypes
            mod.set_axon_ntff_profile_hook(
                _ntff_profile_via_ctypes('/opt/axon/libaxon_pjrt.so'))
    except Exception:
        pass


_install_hook()

import concourse.bass as bass
import concourse.mybir as mybir
from concourse.tile import TileContext
from concourse import bass_utils

AF = mybir.ActivationFunctionType
OP = mybir.AluOpType
dt = mybir.dt

B, H, W = 4, 480, 640
PROP = 6
NCORE = 8
ROWS, BW, NB = 512, 336, 4
XF = NB * BW             # 1344
XA, XB = 1, 335          # conv/C-build col range within a block
XW = XB - XA             # 334
X2A, X2B = 2, 334        # own-write col range per block
XO = X2B - X2A           # 332
XL = XF - 4              # 1340: iteration op width, reads cover [0, XF)
NT, TCH = 42, 12         # triples per (cv, block); triples per slab chunk
F16, F32, BF16 = dt.float16, dt.float32, dt.bfloat16

SH = [(1, 1), (1, 0), (1, -1), (0, 1), (0, -1), (-1, 1), (-1, 0), (-1, -1)]
TAPS = [j for j in range(9) if j != 4]

GRP = {}
for _i in range(3):
    _r0, _nr = {0: (1, 2), 1: (0, 3), 2: (0, 2)}[_i]
    for _j in range(3):
        _c0, _nc = {0: (1, 2), 1: (0, 3), 2: (0, 2)}[_j]
        GRP[(_i, _j)] = (3 * _r0 + _c0, _nr, _nc,
                         3 * _r0 + _c0 + 3 * (_i - 1) + (_j - 1))


def _tapview(t, n, i, t2s, nr, nc_, c0, w):
    v = t[0:n, i, t2s:t2s + 1, c0:c0 + w].unsqueeze(1)
    v.ap[1] = [3 * BW, nr]
    v.ap[2] = [BW, nc_]
    return v


def _cellview(t, n, cell0, nr, nc_, c0, w):
    v = t[0:n, cell0:cell0 + 1, c0:c0 + w].unsqueeze(1)
    v.ap[1] = [3 * XF, nr]
    v.ap[2] = [XF, nc_]
    return v


def _win3(t, n, c0, width):
    v = t[0:n, c0:c0 + width].unsqueeze(1)
    v.ap[1] = [1, 3]
    return v


def _blocks(v):
    assert v.ap[-1][0] == 1 and v.ap[-1][1] == XF
    v = v.unsqueeze(len(v.ap) - 1)
    v.ap[-2] = [BW, NB]
    v.ap[-1] = [1, BW]
    return v


def _split_2d_f16(nc):
    nsp = 0
    for f in nc.m.functions:
        for blk in f.blocks:
            for inst in blk.instructions:
                if type(inst).__name__ not in (
                        "InstTensorTensor", "InstTensorCopy",
                        "InstActivation", "InstTensorScalarPtr"):
                    continue
                for arg in list(inst.ins) + list(inst.outs):
                    ap = getattr(arg, 'ap', None)
                    dtp = getattr(arg, 'dtype', None)
                    if ap is None or dtp is None:
                        continue
                    try:
                        dsz = mybir.dt.np(dtp)().itemsize
                    except Exception:
                        continue
                    if (dsz == 2 and len(ap) == 2 and ap[1][0] == 1
                            and ap[1][1] % 2 == 0 and ap[1][1] >= 2):
                        n = ap[1][1]
                        arg.ap = [list(ap[0]), [n // 2, 2], [1, n // 2]]
                        nsp += 1
    return nsp


def _split_waits(nc, maxw=1):
    n_split = 0
    for f in nc.m.functions:
        for blk in f.blocks:
            out_list = []
            changed = False
            for inst in blk.instructions:
                si = inst.sync_info
                if si is not None and len(si.on_wait) > maxw:
                    waits = list(si.on_wait)
                    extra, keep = waits[:-maxw], waits[-maxw:]
                    for w_i, w in enumerate(extra):
                        nop = mybir.InstNoOp(name=f"{inst.name}-w{w_i}",
                                             ins=[], outs=[])
                        nop.engine = inst.engine
                        nop.sync_info = mybir.SyncInfo(on_wait=[w], on_update=[])
                        out_list.append(nop)
                        n_split += 1
                    si.on_wait = keep
                    inst.sync_info = si
                    changed = True
                out_list.append(inst)
            if changed:
                blk.instructions = out_list
    return n_split


def _pack_conv(w, bi):
    """[121, 81] stationary: contraction (c,j,d) planes + bias plane 120.
    Out field = s*27 + q*9 + t2 for triple row s."""
    Wm = np.zeros((121, 81), np.float32)
    for s in range(3):
        for t2 in range(9):
            if t2 == 4:
                continue
            idx = TAPS.index(t2)
            for q in range(3):
                oref = 2 * idx if q == 0 else (2 * idx + 1 if q == 1
                                               else 16 + idx)
                o81 = s * 27 + q * 9 + t2
                Wm[120, o81] = bi[oref]
                for d in range(3):
                    for c in range(8):
                        for ky in range(3):
                            j = s + ky
                            Wm[(c * 5 + j) * 3 + d, o81] = w[oref, c, ky, d]
    return Wm


def build_nc():
    nc = bass.Bass(trn_type="TRN2")
    for val in (1e-4,):
        _t = nc.alloc_sbuf_tensor(f"const-f32-{val}", [128, 1], F32)
        nc.gpsimd.memset(_t.ap(), val)
        nc.const_aps.aps[(F32, val)] = _t.ap()
    nc.all_engine_barrier()
    gD = nc.dram_tensor("g", [16, ROWS, BW], BF16, kind="ExternalInput")
    g3D = nc.dram_tensor("g3", [8, ROWS, BW], F16, kind="ExternalInput")
    dyD = nc.dram_tensor("dyn", [24, ROWS, BW], F16, kind="ExternalInput")
    fiD = nc.dram_tensor("fin", [ROWS, BW], F16, kind="ExternalInput")
    cfD = nc.dram_tensor("cnf", [ROWS, BW], F16, kind="ExternalInput")
    fxD = nc.dram_tensor("ffx", [ROWS, BW], F16, kind="ExternalInput")
    w1D = nc.dram_tensor("w1", [121, 81], F32, kind="ExternalInput")
    shD = nc.dram_tensor("shm", [2, 128, 128], F32, kind="ExternalInput")
    w2D = nc.dram_tensor("w2", [121, 81], F32, kind="ExternalInput")
    outD = nc.dram_tensor("out", [480, 332], F16, kind="ExternalOutput")
    eD = nc.dram_tensor("erp", [PROP, 124, 6, NB, BW], F16)

    qcnt = [0]

    def dmaq(n=2):
        # rotate DMA issue over SP / ACT (/ Pool SWDGE when n=3)
        qcnt[0] += 1
        return (nc.sync, nc.scalar, nc.gpsimd)[qcnt[0] % n]

    def _rows124(dram2d, off=0, ch=None):
        base = dram2d if ch is None else dram2d[ch]
        v = base[2 + off:126 + off, 0:BW].unsqueeze(1)
        v.ap[1] = [124 * BW, NB]
        return v

    with TileContext(nc) as tc:
        with tc.tile_pool(name="outer", bufs=1) as po:
            C1 = po.tile([128, 9, XF], F16, tag="C1", name="C1")
            C2 = po.tile([128, 9, XF], F16, tag="C2", name="C2")
            betT = po.tile([128, XF], F16, tag="betT", name="betT")
            finT = po.tile([128, XF], F16, tag="finT", name="finT")
            alpT = po.tile([128, XF], F16, tag="alpT", name="alpT")
            Afl = po.tile([128, 6, XF], F16, tag="Afl", name="Afl")
            wB = [po.tile([121, 81], BF16, tag=f"wB{cv}", name=f"wB{cv}")
                  for cv in range(2)]
            shT = [po.tile([128, 128], F16, tag=f"sh{z}", name=f"sh{z}")
                   for z in range(2)]

            # ---- input loads (no casts: host staged dtypes) ----
            dmaq().dma_start(out=_blocks(finT[0:124, 0:XF]), in_=_rows124(fiD))
            for cv, wD in enumerate((w1D, w2D)):
                nc.gpsimd.dma_start(out=wB[cv][:, :], in_=wD[:, :])
            for z in range(2):
                nc.gpsimd.dma_start(out=shT[z][:, :], in_=shD[z, :, :])

            with tc.tile_pool(name="prep0", bufs=1) as pp0:
                cnfT = pp0.tile([128, XF], F16, tag="cnfT")
                ffxT = pp0.tile([128, XF], F16, tag="ffxT")
                dmaq().dma_start(out=_blocks(cnfT[0:124, 0:XF]),
                                 in_=_rows124(cfD))
                dmaq().dma_start(out=_blocks(ffxT[0:124, 0:XF]),
                                 in_=_rows124(fxD))
                nc.scalar.activation(out=alpT[0:124, :], in_=cnfT[0:124, :],
                                     func=AF.Sigmoid)
                nc.scalar.activation(out=cnfT[0:124, :], in_=ffxT[0:124, :],
                                     func=AF.Sign)
                nc.vector.tensor_tensor(out=alpT[0:124, :], in0=alpT[0:124, :],
                                        in1=cnfT[0:124, :], op=OP.mult)
                nc.vector.tensor_tensor(out=betT[0:124, :], in0=alpT[0:124, :],
                                        in1=ffxT[0:124, :], op=OP.mult)
                nc.vector.tensor_scalar(out=alpT[0:124, :], in0=alpT[0:124, :],
                                        scalar1=-1.0, scalar2=1.0,
                                        op0=OP.mult, op1=OP.add)

            # ================= phase B: conv + C build =================
            with tc.tile_pool(name="pre1", bufs=1) as p1, \
                 tc.tile_pool(name="pre2", bufs=2) as p2, \
                 tc.tile_pool(name="psum", bufs=2, space="PSUM") as pps:
                # slab tiles: plane 120 stays 1.0 (bias row of the
                # contraction); loads only touch planes 0..119
                sls = [p1.tile([121, TCH, BW], BF16, tag=f"sl{z}",
                               name=f"sl{z}") for z in range(2)]
                for z in range(2):
                    nc.gpsimd.memset(sls[z][:, :, :], 1.0)
                ry = p1.tile([128, 3, 9, BW], F16, tag="ry")
                wx = p1.tile([128, 3, 9, BW], F16, tag="wx")
                tmpP = p1.tile([128, 3, 3, BW], F16, tag="tmpP")

                def _p9(pl, npl, c0=XA, w=XW):
                    v = tmpP[0:124, pl // 3, pl % 3, c0:c0 + w].unsqueeze(1)
                    v.ap[1] = [BW, npl]
                    return v
                zchunk = [0]
                for ib in range(NB):
                    for cv in range(2):
                        Ct = C1 if cv == 0 else C2
                        oa = p2.tile([128, 27, BW], F16, tag="oa")
                        for tc0 in range(0, NT, TCH):
                            csz = min(TCH, NT - tc0)
                            sl = sls[zchunk[0] % 2]
                            zchunk[0] += 1
                            for j in range(5):
                                for d in range(3):
                                    r0 = 124 * ib + 3 * tc0 + j + 1
                                    v = gD[8 * cv:8 * cv + 8,
                                           r0:r0 + 1, d:BW]
                                    v.ap[1] = [3 * BW, csz]
                                    dmaq().dma_start(
                                        out=sl[(j * 3 + d):120:15, 0:csz,
                                               0:BW - d],
                                        in_=v)
                            for tg in range(0, csz, 4):
                                ng = min(4, csz - tg)
                                pt = pps.tile([81, 4, 512], F32, tag="pt")
                                for ti in range(ng):
                                    nc.tensor.matmul(
                                        pt[:, ti, 0:XW], wB[cv][:, :],
                                        sl[:, tg + ti, 0:XW],
                                        start=True, stop=True)
                                et = p2.tile([81, 4, XW], F16, tag="et")
                                pv = pt[:, 0:ng, 0:XW]
                                if tg % 8 == 0:
                                    nc.scalar.activation(out=et[:, 0:ng, :],
                                                         in_=pv, func=AF.Copy)
                                else:
                                    nc.vector.tensor_copy(out=et[:, 0:ng, :],
                                                          in_=pv)
                                for ti in range(ng):
                                    t = tc0 + tg + ti
                                    nc.gpsimd.dma_start(
                                        out=oa[3 * t:3 * t + 3, 0:27, XA:XB],
                                        in_=et[:, ti, :])
                        # ---- C build (truncated 3x3) ----
                        ty = oa[0:124, 0:9, XA:XB]
                        tx = oa[0:124, 9:18, XA:XB]
                        mv = oa[0:124, 18:27, XA:XB]
                        nc.scalar.activation(out=ry[0:124, 2, :, XA:XB],
                                             in_=ty, func=AF.Relu)
                        nc.scalar.activation(out=ry[0:124, 0, :, XA:XB],
                                             in_=ty, func=AF.Relu, scale=-1.0)
                        nc.vector.tensor_tensor(out=ry[0:124, 1, :, XA:XB],
                                                in0=ry[0:124, 2, :, XA:XB],
                                                in1=ry[0:124, 0, :, XA:XB],
                                                op=OP.add)
                        nc.vector.tensor_scalar(out=ry[0:124, 1, :, XA:XB],
                                                in0=ry[0:124, 1, :, XA:XB],
                                                scalar1=-1.0, scalar2=1.0,
                                                op0=OP.mult, op1=OP.add)
                        for i in range(3):
                            nc.vector.tensor_tensor(out=ry[0:124, i, :, XA:XB],
                                                    in0=ry[0:124, i, :, XA:XB],
                                                    in1=mv, op=OP.mult)
                        nc.scalar.activation(out=wx[0:124, 2, :, XA:XB],
                                             in_=tx, func=AF.Relu)
                        nc.scalar.activation(out=wx[0:124, 0, :, XA:XB],
                                             in_=tx, func=AF.Relu, scale=-1.0)
                        nc.vector.tensor_tensor(out=wx[0:124, 1, :, XA:XB],
                                                in0=wx[0:124, 2, :, XA:XB],
                                                in1=wx[0:124, 0, :, XA:XB],
                                                op=OP.add)
                        nc.vector.tensor_scalar(out=wx[0:124, 1, :, XA:XB],
                                                in0=wx[0:124, 1, :, XA:XB],
                                                scalar1=-1.0, scalar2=1.0,
                                                op0=OP.mult, op1=OP.add)
                        t2s, nr, nc_, cell0 = GRP[(1, 1)]
                        nc.vector.tensor_tensor(
                            out=_cellview(Ct, 124, cell0, nr, nc_,
                                          ib * BW + XA, XW),
                            in0=_tapview(ry, 124, 1, t2s, nr, nc_, XA, XW),
                            in1=_tapview(wx, 124, 1, t2s, nr, nc_, XA, XW),
                            op=OP.mult)
                        for (i, j) in ((0, 0), (0, 1), (0, 2), (1, 0), (1, 2),
                                       (2, 0), (2, 1), (2, 2)):
                            t2s, nr, nc_, cell0 = GRP[(i, j)]
                            pvv = tmpP[0:124, 0:nr, 0:nc_, XA:XB]
                            nc.vector.tensor_tensor(
                                out=pvv,
                                in0=_tapview(ry, 124, i, t2s, nr, nc_,
                                             XA, XW),
                                in1=_tapview(wx, 124, j, t2s, nr, nc_,
                                             XA, XW),
                                op=OP.mult)
                            nc.vector.tensor_tensor(
                                out=_cellview(Ct, 124, cell0, nr, nc_,
                                              ib * BW + XA, XW),
                                in0=_cellview(Ct, 124, cell0, nr, nc_,
                                              ib * BW + XA, XW),
                                in1=pvv, op=OP.add)
                        # ---- A field capture (m sums) on Pool ----
                        nc.scalar.activation(out=_p9(0, 9), in_=mv,
                                             func=AF.Abs)
                        for (isabs, ch) in ((False, cv), (True, 3 + cv)):
                            if isabs:
                                v03, v47 = _p9(0, 4), _p9(4, 4)
                                v8 = tmpP[0:124, 2, 2, XA:XB]
                            else:
                                v03 = oa[0:124, 18:22, XA:XB]
                                v47 = oa[0:124, 22:26, XA:XB]
                                v8 = oa[0:124, 26, XA:XB]
                            s4 = _p9(0, 4) if isabs else oa[0:124, 18:22,
                                                            XA:XB]
                            nc.vector.tensor_tensor(out=s4, in0=v03, in1=v47,
                                                    op=OP.add)
                            dstv = Afl[0:124, ch, ib * BW + XA:ib * BW + XB]
                            s2a = _p9(0, 2) if isabs else oa[0:124, 18:20,
                                                             XA:XB]
                            s2b = _p9(2, 2) if isabs else oa[0:124, 20:22,
                                                             XA:XB]
                            nc.vector.tensor_tensor(out=s2a, in0=s2a, in1=s2b,
                                                    op=OP.add)
                            p0 = (tmpP[0:124, 0, 0, XA:XB] if isabs
                                  else oa[0:124, 18, XA:XB])
                            p1_ = (tmpP[0:124, 0, 1, XA:XB] if isabs
                                   else oa[0:124, 19, XA:XB])
                            nc.vector.tensor_tensor(out=dstv, in0=p0, in1=p1_,
                                                    op=OP.add)
                            nc.vector.tensor_tensor(out=dstv, in0=dstv,
                                                    in1=v8, op=OP.add)
                            if isabs:
                                nc.vector.tensor_scalar(
                                    out=dstv, in0=dstv, scalar1=1e-4,
                                    scalar2=None, op0=OP.add)
                # zero C rows >= 480 (block 3, p >= 108)
                nc.gpsimd.memset(tmpP[:, :, :, :], 0.0)
                for Ct in (C1, C2):
                    nc.sync.dma_start(out=Ct[108:128, :, 3 * BW:XF],
                                      in_=tmpP[0:20, :, :, :])

            # F row-variants and shifted g3 (allocated after phase B so
            # its pools could use the space)
            Fv = [[po.tile([128, XF], F16, tag=f"F{s}{d}", name=f"F{s}{d}")
                   for d in range(3)] for s in range(2)]
            g3T = po.tile([128, 8, XF], F16, tag="g3T", name="g3T")
            nc.gpsimd.memset(g3T[:, :, :], 0.0)
            for s in range(2):
                for d2 in range(3):
                    nc.gpsimd.memset(Fv[s][d2][:, :], 0.0)
            for s in range(2):
                for d in range(3):
                    dmaq().dma_start(out=_blocks(Fv[s][d][0:124, 0:XF]),
                                     in_=_rows124(fiD, off=d - 1))
            for ch, (sdy, sdx) in enumerate(SH):
                # bake the tap shift: g3T[p, ch, x] = g3(124b+p+sdy, x+sdx)
                c0, c1 = max(0, sdx), min(BW, BW + sdx)
                v = g3D[ch][2 + sdy:126 + sdy, c0:c1].unsqueeze(1)
                v.ap[1] = [124 * BW, NB]
                ov = g3T[0:124, ch,
                         c0 - sdx:c0 - sdx + (c1 - c0)].unsqueeze(1)
                ov.ap[1] = [BW, NB]
                dmaq().dma_start(out=ov, in_=v)

            # ================= phase C: gate fields =================
            with tc.tile_pool(name="ebp", bufs=2) as pe:
                tmpX = pe.tile([128, XF], F16, tag="tmpX")
                for k6, ab in ((2, False), (5, True)):
                    first = True
                    for ch in range(8):
                        gv = g3T[0:124, ch, 2:2 + XL]
                        dstv = Afl[0:124, k6, 2:2 + XL]
                        if ab:
                            nc.scalar.activation(out=tmpX[0:124, 2:2 + XL],
                                                 in_=gv, func=AF.Abs)
                            gv = tmpX[0:124, 2:2 + XL]
                        if first:
                            nc.vector.tensor_copy(out=dstv, in_=gv)
                            first = False
                        else:
                            nc.vector.tensor_tensor(out=dstv, in0=dstv,
                                                    in1=gv, op=OP.add)
                    if ab:
                        nc.vector.tensor_scalar(out=Afl[0:124, k6, 2:2 + XL],
                                                in0=Afl[0:124, k6, 2:2 + XL],
                                                scalar1=1e-4, scalar2=None,
                                                op0=OP.add)
                for g in range(3):
                    nc.vector.tensor_tensor(out=Afl[0:124, g, :],
                                            in0=Afl[0:124, 3 + g, :],
                                            in1=Afl[0:124, g, :],
                                            op=OP.subtract)
                for k in range(PROP):
                    E = pe.tile([128, 6, XF], F16, tag="Ek")
                    Pt = pe.tile([128, XF], F16, tag="Pt")
                    Tt = pe.tile([128, XF], F16, tag="Tt")
                    T2 = pe.tile([128, XF], F16, tag="T2")
                    Pf = pe.tile([128, XF], F32, tag="Pf")
                    Rf = pe.tile([128, XF], F32, tag="Rf")
                    for g in range(4):
                        dmaq().dma_start(out=_blocks(E[0:124, g, 0:XF]),
                                         in_=_rows124(dyD, ch=4 * k + g))
                    nc.scalar.activation(out=E[0:124, 0:4, :],
                                         in_=E[0:124, 0:4, :], func=AF.Exp)
                    Pv, Tv, T2v = Pt[0:124, :], Tt[0:124, :], T2[0:124, :]
                    nc.vector.tensor_tensor(out=Pv, in0=E[0:124, 0, :],
                                            in1=Afl[0:124, 3, :], op=OP.mult)
                    for g in (1, 2):
                        nc.vector.tensor_tensor(out=Tv, in0=E[0:124, g, :],
                                                in1=Afl[0:124, 3 + g, :],
                                                op=OP.mult)
                        nc.vector.tensor_tensor(out=Pv, in0=Pv, in1=Tv,
                                                op=OP.add)
                    nc.vector.tensor_scalar(out=Tv, in0=E[0:124, 3, :],
                                            scalar1=1.0 + 1e-4, scalar2=None,
                                            op0=OP.mult)
                    nc.vector.tensor_tensor(out=Pv, in0=Pv, in1=Tv, op=OP.add)
                    nc.scalar.activation(out=Pf[0:124, :], in_=Pv,
                                         func=AF.Copy)
                    nc.vector.reciprocal(out=Rf[0:124, :], in_=Pf[0:124, :])
                    nc.vector.tensor_tensor(out=E[0:124, 4, :],
                                            in0=Rf[0:124, :],
                                            in1=alpT[0:124, :], op=OP.mult)
                    nc.vector.tensor_tensor(out=T2v, in0=E[0:124, 0, :],
                                            in1=Afl[0:124, 0, :], op=OP.mult)
                    for g in (1, 2):
                        nc.vector.tensor_tensor(out=Tv, in0=E[0:124, g, :],
                                                in1=Afl[0:124, g, :],
                                                op=OP.mult)
                        nc.vector.tensor_tensor(out=T2v, in0=T2v, in1=Tv,
                                                op=OP.add)
                    nc.vector.tensor_scalar(out=Tv, in0=E[0:124, 3, :],
                                            scalar1=1e-4, scalar2=None,
                                            op0=OP.mult)
                    nc.vector.tensor_tensor(out=T2v, in0=T2v, in1=Tv,
                                            op=OP.add)
                    nc.vector.tensor_tensor(out=E[0:124, 5, :], in0=T2v,
                                            in1=finT[0:124, :], op=OP.mult)
                    for z, eng in ((0, nc.sync), (2, nc.scalar),
                                   (4, nc.gpsimd)):
                        eng.dma_start(
                            out=eD[k, 0:124, z:z + 2, 0:NB, 0:BW],
                            in_=E[0:124, z:z + 2, 0:XF])

            tc.strict_bb_all_engine_barrier()

            # ================= iterations =================
            with tc.tile_pool(name="it1", bufs=1) as i1, \
                 tc.tile_pool(name="it2", bufs=2) as i2, \
                 tc.tile_pool(name="it3", bufs=1) as i3, \
                 tc.tile_pool(name="itps", bufs=1, space="PSUM") as ips:
                u1 = i1.tile([128, XF], F16, tag="u1")
                u2 = i1.tile([128, XF], F16, tag="u2")
                u2b = i1.tile([128, XF], F16, tag="u2b")
                u3 = i1.tile([128, XF], F16, tag="u3")
                u3p = i1.tile([128, XF], F16, tag="u3p")
                num = i1.tile([128, XF], F16, tag="num")
                cmb = i1.tile([128, XF], F16, tag="cmb")
                prodp = i3.tile([128, 3, XF], F16, tag="prodp")
                for k in range(PROP):
                    Fc = Fv[k % 2]
                    Fn = Fv[(k + 1) % 2]
                    itf = i2.tile([128, 6, XF], F16, tag="itf")
                    for z, eng in ((0, nc.sync), (2, nc.scalar),
                                   (4, nc.gpsimd)):
                        eng.dma_start(out=itf[0:124, z:z + 2, 0:XF],
                                      in_=eD[k, 0:124, z:z + 2, 0:NB, 0:BW])
                    # --- Pool subtree: u2b (C2 row-group 2) + u3p (ch 5..7)
                    nc.gpsimd.tensor_tensor(
                        out=prodp[0:124, 0:3, 2:2 + XL],
                        in0=C2[0:124, 6:9, 2:2 + XL],
                        in1=_win3(Fc[2], 124, 1, XL), op=OP.mult)
                    nc.gpsimd.tensor_tensor(out=u2b[0:124, 2:2 + XL],
                                            in0=prodp[0:124, 0, 2:2 + XL],
                                            in1=prodp[0:124, 1, 2:2 + XL],
                                            op=OP.add)
                    nc.gpsimd.tensor_tensor(out=u2b[0:124, 2:2 + XL],
                                            in0=u2b[0:124, 2:2 + XL],
                                            in1=prodp[0:124, 2, 2:2 + XL],
                                            op=OP.add)
                    for z, ch in enumerate((5, 6, 7)):
                        sdy, sdx = SH[ch]
                        gv = g3T[0:124, ch, 2:2 + XL]
                        fv = Fc[1 + sdy][0:124, 2 + sdx:2 + sdx + XL]
                        if z == 0:
                            nc.gpsimd.tensor_tensor(out=u3p[0:124, 2:2 + XL],
                                                    in0=gv, in1=fv,
                                                    op=OP.mult)
                        else:
                            cm2v = prodp[0:124, 0, 2:2 + XL]
                            nc.gpsimd.tensor_tensor(out=cm2v, in0=gv, in1=fv,
                                                    op=OP.mult)
                            nc.gpsimd.tensor_tensor(
                                out=u3p[0:124, 2:2 + XL],
                                in0=u3p[0:124, 2:2 + XL],
                                in1=cm2v, op=OP.add)
                    # --- DVE: u1 (3 groups), u2a (2 groups), u3 ch 0..4
                    for (ut, Ct, gis) in ((u1, C1, (0, 1, 2)),
                                          (u2, C2, (0, 1))):
                        uv = ut[0:124, 2:2 + XL]
                        for gi in gis:
                            prod = i2.tile([128, 3, XF], F16, tag="prod")
                            nc.vector.tensor_tensor(
                                out=prod[0:124, 0:3, 2:2 + XL],
                                in0=Ct[0:124, 3 * gi:3 * gi + 3, 2:2 + XL],
                                in1=_win3(Fc[gi], 124, 1, XL),
                                op=OP.mult)
                            ci0 = 0
                            if gi == 0:
                                nc.vector.tensor_tensor(
                                    out=uv, in0=prod[0:124, 0, 2:2 + XL],
                                    in1=prod[0:124, 1, 2:2 + XL], op=OP.add)
                                ci0 = 2
                            for ci in range(ci0, 3):
                                nc.vector.tensor_tensor(
                                    out=uv, in0=uv,
                                    in1=prod[0:124, ci, 2:2 + XL], op=OP.add)
                    first = True
                    for ch in (0, 1, 2, 3, 4):
                        sdy, sdx = SH[ch]
                        gv = g3T[0:124, ch, 2:2 + XL]
                        fv = Fc[1 + sdy][0:124, 2 + sdx:2 + sdx + XL]
                        if first:
                            nc.vector.tensor_tensor(out=u3[0:124, 2:2 + XL],
                                                    in0=gv, in1=fv,
                                                    op=OP.mult)
                            first = False
                        else:
                            nc.vector.tensor_tensor(out=cmb[0:124, 2:2 + XL],
                                                    in0=gv, in1=fv,
                                                    op=OP.mult)
                            nc.vector.tensor_tensor(
                                out=u3[0:124, 2:2 + XL],
                                in0=u3[0:124, 2:2 + XL],
                                in1=cmb[0:124, 2:2 + XL], op=OP.add)
                    # join subtrees
                    nc.vector.tensor_tensor(out=u2[0:124, 2:2 + XL],
                                            in0=u2[0:124, 2:2 + XL],
                                            in1=u2b[0:124, 2:2 + XL],
                                            op=OP.add)
                    nc.vector.tensor_tensor(out=u3[0:124, 2:2 + XL],
                                            in0=u3[0:124, 2:2 + XL],
                                            in1=u3p[0:124, 2:2 + XL],
                                            op=OP.add)
                    # combine
                    NV = num[0:124, 2:2 + XL]
                    CV = cmb[0:124, 2:2 + XL]
                    Ev = [itf[0:124, q, 2:2 + XL] for q in range(6)]
                    nc.vector.tensor_tensor(out=NV, in0=Ev[0],
                                            in1=u1[0:124, 2:2 + XL],
                                            op=OP.mult)
                    nc.vector.tensor_tensor(out=CV, in0=Ev[1],
                                            in1=u2[0:124, 2:2 + XL],
                                            op=OP.mult)
                    nc.vector.tensor_tensor(out=NV, in0=NV, in1=CV, op=OP.add)
                    nc.vector.tensor_tensor(out=CV, in0=Ev[2],
                                            in1=u3[0:124, 2:2 + XL],
                                            op=OP.mult)
                    nc.vector.tensor_tensor(out=NV, in0=NV, in1=CV, op=OP.add)
                    nc.vector.tensor_tensor(out=CV, in0=Ev[3],
                                            in1=Fc[1][0:124, 2:2 + XL],
                                            op=OP.mult)
                    nc.vector.tensor_tensor(out=NV, in0=NV, in1=CV, op=OP.add)
                    nc.vector.tensor_tensor(out=NV, in0=NV, in1=Ev[5],
                                            op=OP.add)
                    nc.vector.tensor_tensor(out=NV, in0=NV, in1=Ev[4],
                                            op=OP.mult)
                    nwv = Fn[1][0:124, X2A:X2A + XO].unsqueeze(1)
                    nwv.ap[1] = [BW, NB]
                    nv_in = num[0:124, X2A:X2A + XO].unsqueeze(1)
                    nv_in.ap[1] = [BW, NB]
                    bt_in = betT[0:124, X2A:X2A + XO].unsqueeze(1)
                    bt_in.ap[1] = [BW, NB]
                    nc.vector.tensor_tensor(out=nwv, in0=nv_in, in1=bt_in,
                                            op=OP.add)
                    if k < PROP - 1:
                        # row-shifted variants via PE shift-matmul
                        for z, dst in ((0, Fn[0]), (1, Fn[2])):
                            psh = ips.tile([128, 3, 512], F32,
                                           tag=f"psh{z}", name=f"psh{z}k{k}")
                            for cpos, cw in ((0, 512), (512, 512),
                                             (1024, XF - 1024)):
                                nc.tensor.matmul(
                                    psh[:, cpos // 512, 0:cw],
                                    shT[z][:, :],
                                    Fn[1][0:128, cpos:cpos + cw],
                                    start=True, stop=True)
                            nc.scalar.activation(out=dst[0:128, 0:1024],
                                                 in_=psh[:, 0:2, 0:512],
                                                 func=AF.Copy)
                            nc.scalar.activation(out=dst[0:128, 1024:XF],
                                                 in_=psh[:, 2, 0:XF - 1024],
                                                 func=AF.Copy)
                        nc.sync.dma_start(out=Fn[0][0:1, BW:XF],
                                          in_=Fn[1][123:124, 0:3 * BW])
                        nc.scalar.dma_start(out=Fn[2][123:124, 0:3 * BW],
                                            in_=Fn[1][0:1, BW:XF])
                for b in range(NB):
                    pend = 108 if b == 3 else 124
                    nc.sync.dma_start(
                        out=outD[124 * b:124 * b + pend, :],
                        in_=Fv[PROP % 2][1][0:pend,
                                            b * BW + X2A:b * BW + X2B])
    _split_2d_f16(nc)
    _split_waits(nc)
    return nc


_NC_CACHE = {}


def _prep_core_inputs(inputs):
    bf16 = mybir.dt.np(BF16)
    f16 = np.float16
    W1 = _pack_conv(inputs['w_off1'], inputs['b_off1'])
    W2 = _pack_conv(inputs['w_off2'], inputs['b_off2'])
    maps = []
    for c in range(NCORE):
        bimg, half = c // 2, c % 2
        gp = np.zeros((24, ROWS, 644), np.float32)
        gp[:, 2:482, 2:642] = inputs['guidance'][bimg]
        dp = np.zeros((24, ROWS, 644), np.float32)
        dp[:, 2:482, 2:642] = inputs['dynamic'][bimg]
        fp = np.zeros((3, ROWS, 644), np.float32)
        fp[0, 2:482, 2:642] = inputs['feat_init'][bimg, 0]
        fp[1, 2:482, 2:642] = inputs['confidence'][bimg, 0]
        fp[2, 2:482, 2:642] = inputs['feat_fix'][bimg, 0]
        xs = 0 if half == 0 else 308
        shm = np.zeros((2, 128, 128), np.float32)
        shm[0] = np.eye(128, k=1)
        shm[1] = np.eye(128, k=-1)
        maps.append({
            "shm": shm,
            "g": np.ascontiguousarray(gp[0:16, :, xs:xs + BW]).astype(bf16),
            "g3": np.ascontiguousarray(gp[16:24, :, xs:xs + BW]).astype(f16),
            "dyn": np.ascontiguousarray(dp[:, :, xs:xs + BW]).astype(f16),
            "fin": np.ascontiguousarray(fp[0, :, xs:xs + BW]).astype(f16),
            "cnf": np.ascontiguousarray(fp[1, :, xs:xs + BW]).astype(f16),
            "ffx": np.ascontiguousarray(fp[2, :, xs:xs + BW]).astype(f16),
            "w1": W1, "w2": W2,
        })
    return maps


def run_cores(inputs, trace=False):
    if 'nc' not in _NC_CACHE:
        _NC_CACHE['nc'] = build_nc()
    nc = _NC_CACHE['nc']
    maps = _prep_core_inputs(inputs)
    res = bass_utils.run_bass_kernel_spmd(nc, maps, core_ids=list(range(NCORE)),
                                          trace=trace)
    out = np.zeros((B, 1, H, W), np.float32)
    for c in range(NCORE):
        bimg, half = c // 2, c % 2
        o = res.results[c]["out"].astype(np.float32)
        if half == 0:
            out[bimg, 0, :, 0:320] = o[:, 0:320]
        else:
            out[bimg, 0, :, 320:640] = o[:, 12:332]
    return out, res


def kernel(**inputs):
    out, _ = run_cores(inputs, trace=False)
    return out


if __name__ == "__main__":
    import pickle
    with open('/tmp/inputs.pkl', 'rb') as f:
        inputs = pickle.load(f)
    ref = np.load('/tmp/ref_out.npy')
    got, res = run_cores(inputs, trace=False)
    rel = np.linalg.norm(got - ref) / np.linalg.norm(ref)
    print("Relative error:", rel, " absmax:", np.abs(got - ref).max())
